# revision 1
# baseline (speedup 1.0000x reference)
"""Trainium2 Bass kernel for CurvSelfAttention.

Reference computation (per batch b):
    Q = hs @ Wq + bq ; K = hs @ Wk + bk ; V = hs @ Wv + bv      # [S, H]
    s = sigmoid(hs @ Ws + bs) * 0.2 + 0.9                        # [S, NH*G]
    Q[:, h*64+g*8+r] *= s[:, h*8+g]
    per head h: ctx_h = softmax(Q_h K_h^T / 8) V_h               # [S, 64]
    out = concat_h(ctx_h)                                        # [S, NH*64]

Sharding over 8 cores: core c = (b = c // 2, hh = c % 2); each core owns
batch b and heads hh*8 .. hh*8+8 (512 output columns). Entirely
data/tensor-parallel - no collectives.

Per-core device algorithm (all matmuls bf16 with fp32 PSUM accumulation):
  hsT[k, t]          <- xbar-DMA-transpose of hs (bf16)
  s^T (compact)      <- Ws^T-stationary matmul, sigmoid*0.2+0.9, then
                        group-replicated x8 via a DRAM bounce
  Q^T, K^T [j, t]    <- W^T-stationary matmuls (j = head*64+d on partitions)
  V [t, j]           <- hsT-chunk-stationary matmuls (natural row layout)
  scores^T[t, s]     =  K^T_h slices as stationary; two heads row-packed
                        (K=64 at array rows 0/64, adjacent -> concurrent)
  probs^T            =  exp(scores^T / 8)   (no max subtraction: |scores|<~6)
  ctx^T[d,s]+denom   =  [V_h | 1]^T @ probs^T; ones column of V gives the
                        softmax denominator as output row 64
  out[s, d]          =  PE-transpose of ctx^T chunks, * 1/denom

Scheduling: most projection matmuls are interleaved INTO the attention
chunk loop as "filler" units. This overlaps the projection phase with the
exp-bound attention stream AND keeps TensorE continuously busy so the PE
HAM clock gate stays at 2.4 GHz (idle gaps re-throttle it to 1.2 GHz,
which measured as a ~2x slowdown on every attention matmul).
"""

import os
import sys

sys.path.insert(0, "/opt/trn_rl_repo")

import numpy as np
import ml_dtypes
from collections import deque
from contextlib import ExitStack

import concourse.bass as bass
import concourse.bacc as bacc
import concourse.tile as tile
from concourse import mybir
from concourse import bass_utils

F32 = mybir.dt.float32
BF16 = mybir.dt.bfloat16
AF = mybir.ActivationFunctionType
ALU = mybir.AluOpType

P = 128          # SBUF partitions
NB = 512         # matmul moving free-dim block
W2 = 1024        # attention s-window (one probs chunk row)
HD = 64          # head dim
G = 8            # groups per head
RING = 4         # probs ring chunks
LAG = 2          # ctx trails exp by this many chunks
SC_MIN, SC_MAX = 0.9, 1.1


def build_bass(S=2048, H=1024, NHL=8):
    """Build the per-core Bass module. NHL = local heads; JL = NHL*64."""
    JL = NHL * HD
    GL = NHL * G           # compact scale channels
    KT = H // P            # contraction k-tiles
    JB = JL // P           # j row-blocks for Q/K (2 heads each)
    NTB = S // NB          # 512-wide t blocks
    NTC = S // P           # 128-wide t chunks
    NSSB = S // W2         # 1024-wide s superblocks
    HP = NHL // 2          # head pairs

    nc = bacc.Bacc(trn_type="TRN2", target_bir_lowering=False, debug=False,
                   num_devices=8)

    hs = nc.dram_tensor("hs", [S, H], BF16, kind="ExternalInput").ap()
    wq = nc.dram_tensor("wq", [H, JL], BF16, kind="ExternalInput").ap()
    wk = nc.dram_tensor("wk", [H, JL], BF16, kind="ExternalInput").ap()
    wv = nc.dram_tensor("wv", [H, JL], BF16, kind="ExternalInput").ap()
    ws = nc.dram_tensor("ws", [H, GL], BF16, kind="ExternalInput").ap()
    bq = nc.dram_tensor("bq", [JL], F32, kind="ExternalInput").ap()
    bk = nc.dram_tensor("bk", [JL], F32, kind="ExternalInput").ap()
    bv = nc.dram_tensor("bv", [JL], F32, kind="ExternalInput").ap()
    bs = nc.dram_tensor("bs", [GL], F32, kind="ExternalInput").ap()
    ident = nc.dram_tensor("ident", [P, P], F32, kind="ExternalInput").ap()
    out = nc.dram_tensor("out", [S, JL], F32, kind="ExternalOutput").ap()
    sxd = nc.dram_tensor("sxd", [GL, S], BF16, kind="Internal").ap()

    with tile.TileContext(nc) as tc, ExitStack() as ctx:
        cpool = ctx.enter_context(tc.tile_pool(name="consts", bufs=1))
        qkpool = ctx.enter_context(tc.tile_pool(name="qk", bufs=1))
        vpool = ctx.enter_context(tc.tile_pool(name="v", bufs=1))
        sxpool = ctx.enter_context(tc.tile_pool(name="sexp", bufs=1))
        hpool = ctx.enter_context(tc.tile_pool(name="hsT", bufs=1))
        wpool = ctx.enter_context(tc.tile_pool(name="wts", bufs=1))
        ppsum = ctx.enter_context(tc.tile_pool(name="ppsum", bufs=2,
                                               space="PSUM"))
        ptmp = ctx.enter_context(tc.tile_pool(name="ptmp", bufs=3))

        # ---- constants (small DMAs on the SW DGE queue) ----
        bq_sb = []
        bk_sb = []
        for jb in range(JB):
            t = cpool.tile([P, 1], F32, tag=f"bq{jb}")
            nc.gpsimd.dma_start(
                t[:], bq[jb * P:(jb + 1) * P].rearrange("(a b) -> a b", b=1))
            bq_sb.append(t)
            t = cpool.tile([P, 1], F32, tag=f"bk{jb}")
            nc.gpsimd.dma_start(
                t[:], bk[jb * P:(jb + 1) * P].rearrange("(a b) -> a b", b=1))
            bk_sb.append(t)
        bs_sb = cpool.tile([GL, 1], F32, tag="bs")
        nc.gpsimd.dma_start(bs_sb[:], bs.rearrange("(a b) -> a b", b=1))
        idf = cpool.tile([P, P], F32, tag="idf")
        nc.gpsimd.dma_start(idf[:], ident)
        bvb = cpool.tile([P, JL], F32, tag="bvb")
        nc.gpsimd.dma_start(
            bvb[:], bv.rearrange("(a b) -> a b", a=1).broadcast_to([P, JL]))

        # persistent activation tensors
        q_sb = [qkpool.tile([P, S], BF16, tag=f"q{jb}", name=f"q{jb}")
                for jb in range(JB)]
        k_sb = [qkpool.tile([P, S], BF16, tag=f"k{jb}", name=f"k{jb}")
                for jb in range(JB)]
        # V as [t-chunk][128, NHL, 65]; col 64 of each head = ones (denom)
        v_sb = [vpool.tile([P, NHL, HD + 1], BF16, tag=f"v{tc_}", name=f"v{tc_}")
                for tc_ in range(NTC)]
        sexp = [sxpool.tile([P, S], BF16, tag=f"sx{jb}", name=f"sx{jb}")
                for jb in range(JB)]

        # ---- input loads: ws first, hsT transposes, then weights ----
        ws_sb = []
        for k in range(KT):
            t = wpool.tile([P, GL], BF16, tag=f"ws{k}", name=f"ws{k}")
            nc.sync.dma_start(t[:], ws[k * P:(k + 1) * P, :])
            ws_sb.append(t)
        hsT = []
        for k in range(KT):
            t = hpool.tile([P, S], BF16, tag=f"hsT{k}", name=f"hsT{k}")
            nc.sync.dma_start_transpose(t[:], hs[:, k * P:(k + 1) * P])
            hsT.append(t)
        wq_sb, wk_sb, wv_sb = [], [], []
        for k in range(KT):
            for name, dram, lst in (("wq", wq, wq_sb), ("wk", wk, wk_sb),
                                    ("wv", wv, wv_sb)):
                t = wpool.tile([P, JL], BF16, tag=f"{name}{k}",
                               name=f"{name}{k}")
                nc.sync.dma_start(t[:], dram[k * P:(k + 1) * P, :])
                lst.append(t)

        # ---- projection work units (4 matmuls each), emitted either in
        # the prefix or interleaved into the attention loop ----
        open_ps = {}

        def emit_qk_unit(jb, kind, tb, half):
            wlist = wq_sb if kind == "q" else wk_sb
            key = (jb, kind, tb)
            if half == 0:
                open_ps[key] = ppsum.tile([P, NB], F32, tag="pp", name="pp")
            ps = open_ps[key]
            kh = KT // 2
            for k in range(kh * half, kh * half + kh):
                nc.tensor.matmul(
                    ps[:], wlist[k][:, jb * P:(jb + 1) * P],
                    hsT[k][:, tb * NB:(tb + 1) * NB],
                    start=(k == 0), stop=(k == KT - 1))
            if half == 1:
                del open_ps[key]
                if kind == "q":
                    nc.vector.scalar_tensor_tensor(
                        q_sb[jb][:, tb * NB:(tb + 1) * NB], ps[:],
                        bq_sb[jb][:], sexp[jb][:, tb * NB:(tb + 1) * NB],
                        ALU.add, ALU.mult)
                else:
                    nc.vector.tensor_scalar_add(
                        k_sb[jb][:, tb * NB:(tb + 1) * NB], ps[:], bk_sb[jb][:])

        def emit_v_unit(tc_, half):
            key = ("v", tc_)
            if half == 0:
                open_ps[key] = ppsum.tile([P, JL], F32, tag="pp", name="pp")
            ps = open_ps[key]
            kh = KT // 2
            for k in range(kh * half, kh * half + kh):
                nc.tensor.matmul(
                    ps[:], hsT[k][:, tc_ * P:(tc_ + 1) * P], wv_sb[k][:],
                    start=(k == 0), stop=(k == KT - 1))
            if half == 1:
                del open_ps[key]
                nc.vector.memset(v_sb[tc_][:, :, HD], 1.0)
                for h in range(NHL):
                    nc.vector.tensor_add(
                        v_sb[tc_][:, h, 0:HD], ps[:, h * HD:(h + 1) * HD],
                        bvb[:, h * HD:(h + 1) * HD])

        # ---- prefix: compact dynamic scale, then Q/K for head pair 0 ----
        sxc = ptmp.tile([GL, S], BF16, tag="sxc", bufs=1)
        for tb in range(NTB):
            ps = ppsum.tile([P, NB], F32, tag="pp", name="pp")
            for k in range(KT):
                nc.tensor.matmul(
                    ps[0:GL, :], ws_sb[k][:],
                    hsT[k][:, tb * NB:(tb + 1) * NB],
                    start=(k == 0), stop=(k == KT - 1))
            sg = ptmp.tile([GL, NB], F32, tag="sig")
            nc.scalar.activation(sg[:], ps[0:GL, :], AF.Sigmoid, bias=bs_sb[:])
            nc.vector.tensor_scalar(
                sxc[:, tb * NB:(tb + 1) * NB], sg[:],
                SC_MAX - SC_MIN, SC_MIN, ALU.mult, ALU.add)
        # replicate groups x8 into per-jb expanded scale tiles via DRAM
        # bounce (SBUF sources cannot partition-broadcast):
        # sexp[jb] rows [hl*64+8g : +8] = sxc row (16jb + 8hl + g) x8
        nc.gpsimd.dma_start(sxd, sxc[:])
        for jb in range(JB):
            for hl in range(2):
                for g in range(G):
                    src_row = 16 * jb + 8 * hl + g
                    nc.gpsimd.dma_start(
                        sexp[jb][hl * HD + G * g:hl * HD + G * g + G, :],
                        sxd[src_row:src_row + 1, :].broadcast_to([G, S]))
        for kind in ("q", "k"):
            for tb in range(NTB):
                for half in range(2):
                    emit_qk_unit(0, kind, tb, half)

        # deferred projection units, drained as attention filler
        projq = deque()
        for tc_ in range(NTC):
            for half in range(2):
                projq.append(("v", tc_, half))
        for jb in range(1, JB):
            for kind in ("q", "k"):
                for tb in range(NTB):
                    for half in range(2):
                        projq.append(("qk", jb, kind, tb, half))

        def drain_proj(n):
            for _ in range(n):
                if not projq:
                    return
                u = projq.popleft()
                if u[0] == "v":
                    emit_v_unit(u[1], u[2])
                else:
                    emit_qk_unit(u[1], u[2], u[3], u[4])

        # ================= attention =================
        # PSUM: s0,s1 (1 bank each) + c0,c1 (2 each) + pp (2x1) = 8 banks
        with tc.tile_pool(name="probs", bufs=1) as prpool, \
             tc.tile_pool(name="asm", bufs=1) as apool, \
             tc.tile_pool(name="spsum", bufs=1, space="PSUM") as spsum, \
             tc.tile_pool(name="cpsum", bufs=1, space="PSUM") as cpsum, \
             tc.tile_pool(name="ctmp", bufs=2) as ctpool, \
             tc.tile_pool(name="rtmp", bufs=2) as rtpool:

            asm = [apool.tile([P, JL], F32, tag=f"asm{ssb}_{i}",
                              name=f"asm{ssb}_{i}")
                   for ssb in range(NSSB) for i in range(W2 // P)]

            # per-pair filler quota (units of 4 matmuls per chunk iter):
            # V must finish during (hp0, ssb0); Q/K(jb) before pair (jb, 0)
            quota = {(0, 0): 2, (0, 1): 1, (1, 0): 1, (1, 1): 1,
                     (2, 0): 1, (2, 1): 1, (3, 0): 0, (3, 1): 0}

            for hp in range(HP):
                for ssb in range(NSSB):
                    nq = quota[(hp, ssb)]
                    pts = [prpool.tile([P, RING, W2], BF16, tag=f"p{i}",
                                       name=f"p{i}") for i in range(2)]
                    cps = [cpsum.tile([P, W2], F32, tag=f"c{i}", name=f"c{i}")
                           for i in range(2)]

                    def ctx_chunk(tcc):
                        for i in range(2):
                            h = hp * 2 + i
                            for sh in range(W2 // NB):
                                nc.tensor.matmul(
                                    cps[i][0:HD + 1, sh * NB:(sh + 1) * NB],
                                    v_sb[tcc][:, h, :],
                                    pts[i][:, tcc % RING, sh * NB:(sh + 1) * NB],
                                    start=(tcc == 0), stop=(tcc == NTC - 1))

                    for tc_ in range(NTC):
                        if tc_ >= LAG:
                            ctx_chunk(tc_ - LAG)
                        drain_proj(nq)
                        for sbh in range(W2 // NB):
                            scol = ssb * W2 + sbh * NB
                            pss = []
                            for i in range(2):
                                r0 = i * HD
                                t = spsum.tile([P, NB], F32, tag=f"s{i}",
                                               name=f"s{i}")
                                nc.tensor.matmul(
                                    t[:],
                                    k_sb[hp][r0:r0 + HD, tc_ * P:(tc_ + 1) * P],
                                    q_sb[hp][r0:r0 + HD, scol:scol + NB],
                                    start=True, stop=True)
                                pss.append(t)
                            for i in range(2):
                                nc.scalar.activation(
                                    pts[i][:, tc_ % RING,
                                           sbh * NB:(sbh + 1) * NB],
                                    pss[i][:], AF.Exp, scale=1.0 / 8.0)
                    for tcc in range(NTC - LAG, NTC):
                        ctx_chunk(tcc)
                    # normalize + transpose back per head (PE transposes
                    # time-share the projection psum slots)
                    for i in range(2):
                        h = hp * 2 + i
                        ct = ctpool.tile([HD + 1, W2], F32, tag="ct")
                        nc.vector.tensor_copy(ct[:], cps[i][0:HD + 1, :])
                        for cc in range(W2 // P):
                            trp = ppsum.tile([P, NB], F32, tag="pp",
                                             name="trp")
                            trv = trp[:, 0:HD + 1]
                            nc.tensor.transpose(
                                trv, ct[:, cc * P:(cc + 1) * P],
                                idf[0:HD + 1, 0:HD + 1])
                            rc = rtpool.tile([P, 1], F32, tag="rc")
                            nc.vector.reciprocal(rc[:], trv[:, HD:HD + 1])
                            nc.vector.tensor_scalar_mul(
                                asm[ssb * (W2 // P) + cc][:, h * HD:(h + 1) * HD],
                                trv[:, 0:HD], rc[:])
                    if hp == HP - 1:
                        for i in range(W2 // P):
                            srow = ssb * W2 + i * P
                            nc.sync.dma_start(out[srow:srow + P, :],
                                              asm[ssb * (W2 // P) + i][:])

    nc.finalize()
    return nc


_CACHE = {}


def _get_nc():
    if "nc" not in _CACHE:
        _CACHE["nc"] = build_bass()
    return _CACHE["nc"]


def _shard(inputs):
    """Split full inputs into 8 per-core input maps (host-side, bf16 cast)."""
    hidden_states = inputs["hidden_states"]
    Wq, bq = inputs["Wq"], inputs["bq"]
    Wk, bk = inputs["Wk"], inputs["bk"]
    Wv, bv = inputs["Wv"], inputs["bv"]
    Ws, bs = inputs["Ws"], inputs["bs"]
    JL = 512   # output cols per core
    GL = 64    # Ws cols per core
    bf = ml_dtypes.bfloat16
    ident = np.eye(P, dtype=np.float32)
    in_maps = []
    for c in range(8):
        b, hh = c // 2, c % 2
        in_maps.append({
            "hs": np.ascontiguousarray(hidden_states[b]).astype(bf),
            "wq": np.ascontiguousarray(Wq[:, hh * JL:(hh + 1) * JL]).astype(bf),
            "wk": np.ascontiguousarray(Wk[:, hh * JL:(hh + 1) * JL]).astype(bf),
            "wv": np.ascontiguousarray(Wv[:, hh * JL:(hh + 1) * JL]).astype(bf),
            "ws": np.ascontiguousarray(Ws[:, hh * GL:(hh + 1) * GL]).astype(bf),
            "bq": np.ascontiguousarray(bq[hh * JL:(hh + 1) * JL]).astype(np.float32),
            "bk": np.ascontiguousarray(bk[hh * JL:(hh + 1) * JL]).astype(np.float32),
            "bv": np.ascontiguousarray(bv[hh * JL:(hh + 1) * JL]).astype(np.float32),
            "bs": np.ascontiguousarray(bs[hh * GL:(hh + 1) * GL]).astype(np.float32),
            "ident": ident,
        })
    return in_maps


def kernel(hidden_states, Wq, bq, Wk, bk, Wv, bv, Ws, bs):
    B, S, H = hidden_states.shape
    NH = 16
    JL = 512
    nc = _get_nc()
    in_maps = _shard(dict(hidden_states=hidden_states, Wq=Wq, bq=bq, Wk=Wk,
                          bk=bk, Wv=Wv, bv=bv, Ws=Ws, bs=bs))

    res = bass_utils.run_bass_kernel_spmd(nc, in_maps, core_ids=list(range(8)))

    outp = np.zeros((B, S, NH * HD), dtype=np.float32)
    for c in range(8):
        b, hh = c // 2, c % 2
        outp[b][:, hh * JL:(hh + 1) * JL] = res.results[c]["out"]
    return outp

